# revision 34
# baseline (speedup 1.0000x reference)
"""Trainium2 Bass kernel for nn_AttentionBlock (B=32, C=256, H=W=32).

Data-parallel over batch across 8 NeuronCores (4 batch elements per core);
all parameters replicated.

Algorithm per batch element (x: [C=256, N=1024]):
  h  = GroupNorm(x; 8 groups) * gn_w + gn_b
  q  = (wq/sqrt(C)) @ h + bq/sqrt(C)          [C, N]   (scale folded into wq)
  k  = wk @ h + bk                            [C, N]
  vT = hT @ wvT + 1 x bv                      [N, C]   (produced transposed!)
  ST[j,i] = sum_c k[c,j] q[c,i]               [N, N]   (scores, transposed)
  E  = exp(ST)            (scores are in [-9, 9] for this model; no max-sub)
  rowsum[i] = sum_j E[j,i]                    (ones-vector matmul, PSUM accum)
  outU[c,i] = sum_j vT[j,c] E[j,i]            (PSUM accum over j-tiles)
  y  = x + wp @ (outU * (1/rowsum)) + bp

The transposed-score formulation means no [N,N] transposes are needed:
softmax reductions over j happen on the TensorEngine partition axis via
ones/indicator matmuls. All big matmuls run in bf16 (1 cycle/row, FWL
weight loads; fp32r measures 2 cycles/row on HW), with fp32 PSUM
accumulation throughout; the rowsum is replicated across all 128
partitions by an all-ones stationary operand so the softmax reciprocal
runs wide on the VectorEngine with no partition broadcast.

Emission order is tuned for the in-order per-engine streams: all four
GroupNorm heads are hoisted to the start (clusters ACT Sqrt table loads
away from the Exp table; a dummy Sqrt preloads the table before x even
lands), each batch's qkv projections are emitted between the previous
batch's attention i-halves so the TensorEngine always has matmul work
while DVE/ACT normalization chains run, and the attention j-loop is
software-pipelined by one step (accumulation of tile j issues while
exp of j+1 runs on the ScalarEngine). DMA descriptor issues (~0.7us
each, serialized per issuing engine) are spread across the Sync (x),
Scalar (weights), and GpSimd (packed small constants) queues so the
first matmul fires ~13us in instead of ~33us.
Measured on 8 axon TRN2 cores: ~162us HW exec (~120us TensorE-active),
scale-relative absmax error 2.8e-3 vs a float64 reference.
"""

import numpy as np

import concourse.bacc as bacc
import concourse.bass as bass
import concourse.mybir as mybir
import concourse.tile as tile
from concourse.tile_rust import add_dep_helper
from concourse.bass_utils import run_bass_kernel_spmd

B, C, HH, WW = 32, 256, 32, 32
N = HH * WW                 # 1024 spatial positions
NCORES = 8
BPC = B // NCORES           # batch elements per core
G = 8                       # groupnorm groups
GS = C // G                 # channels per group
P = 128                     # SBUF partitions
NCH = C // P                # channel chunks (2)
IH = 512                    # i-half width (fp32 moving-operand max)
NIH = N // IH               # 2
NJ = N // P                 # 8 j-tiles
EPS = 1e-5

F32 = mybir.dt.float32
F32R = mybir.dt.float32r
BF16 = mybir.dt.bfloat16
# SIG: groupnorm output h, q/k and their weights (drives score precision)
# VAL: exp(S), vT, normalized out, wp weights (value path)
SIG_DT = BF16
VAL_DT = BF16
# j-loop emission: False = interleaved (S_j, exp_j, accum_j);
# True = software-pipelined by one (accum_{j-1} issues after S_j/exp_j)
PIPELINE_J = True
# WIDE_ACC: pair both i-halves per j, accumulate 1024-wide one j behind
WIDE_ACC = False
AF = mybir.ActivationFunctionType
OP = mybir.AluOpType


def r(ap):
    """Matmul-operand APs pass straight through (kept as a seam for dtype
    experiments — bitcasts would go here)."""
    return ap


def build_kernel_body(nc, tc, x_d, y_d, wd, bvr_d, spack_d, indT_d, ones_d):
    ctxpools = dict(
        const=tc.tile_pool(name="const", bufs=1),
        xp=tc.tile_pool(name="xp", bufs=1),
        hp=tc.tile_pool(name="hp", bufs=4),
        qk=tc.tile_pool(name="qk", bufs=3),
        vtp=tc.tile_pool(name="vtp", bufs=3),
        etp=tc.tile_pool(name="etp", bufs=2),
        sm=tc.tile_pool(name="sm", bufs=4),
        outp=tc.tile_pool(name="outp", bufs=2),
        pp=tc.tile_pool(name="pp", bufs=(2 if WIDE_ACC else 8), space=bass.MemorySpace.PSUM),
    )
    pools = {k: v.__enter__() for k, v in ctxpools.items()}
    const = pools["const"]
    pp = pools["pp"]
    sm = pools["sm"]

    # ---- input + constant loads, spread across issue queues ----
    # The DMA descriptor issue costs ~0.7us each and serializes per engine;
    # x goes first on Sync (unblocks GroupNorm), weights on Scalar, packed
    # small constants on GpSimd, so the kernel ramps in ~6us instead of ~30.
    st = {}   # per-batch tiles: xt, ht, qt, kt, vt, fin
    for b in range(BPC):
        xt = []
        for ch in range(NCH):
            t = pools["xp"].tile([P, N], F32, name=f"xt{b}_{ch}", tag=f"xt{b}_{ch}")
            if b == 0:
                # halves: GroupNorm's first bn_stats starts ~1.5us earlier
                for hh in range(2):
                    nc.sync.dma_start(out=t[:, hh * IH:(hh + 1) * IH],
                                      in_=x_d[b, ch * P:(ch + 1) * P, hh * IH:(hh + 1) * IH])
            else:
                nc.sync.dma_start(out=t, in_=x_d[b, ch * P:(ch + 1) * P, :])
            xt.append(t)
        st[b] = dict(xt=xt)

    wt = {}   # weights, transposed: [c_chunk][128, 256]
    for name in ("q", "k", "v", "p"):
        wt[name] = []
        for ch in range(NCH):
            wdt = VAL_DT if name == "p" else SIG_DT
            w_tile = const.tile([P, C], wdt, tag=f"w{name}{ch}")
            nc.scalar.dma_start(out=w_tile, in_=wd[name][ch * P:(ch + 1) * P, :])
            wt[name].append(w_tile)
    ones128 = const.tile([P, P], VAL_DT, tag="ones128")
    nc.scalar.dma_start(out=ones128, in_=ones_d[:, :])
    ones_row = ones128[0:1, :]

    # one packed DMA for all per-partition scalars + group indicators:
    # cols 0-5 = bq0,bq1,bk0,bk1,bp0,bp1; 6-7 gnw; 8-9 gnb; 10-25 ind chunks
    spack = const.tile([P, 26], F32, tag="spack")
    nc.gpsimd.dma_start(out=spack, in_=spack_d[:, :])
    bt = {"q": [spack[:, 0:1], spack[:, 1:2]],
          "k": [spack[:, 2:3], spack[:, 3:4]],
          "p": [spack[:, 4:5], spack[:, 5:6]]}
    gnw_t = [spack[:, 6:7], spack[:, 7:8]]
    gnb_t = [spack[:, 8:9], spack[:, 9:10]]
    ind_t = [spack[:, 10:18], spack[:, 18:26]]

    bv_row = const.tile([1, C], VAL_DT, tag="bv_row")
    nc.gpsimd.dma_start(out=bv_row, in_=bvr_d[None, :])
    indT_t = []
    for ch in range(NCH):
        itT = const.tile([G, P], F32, tag=f"indT{ch}")
        nc.gpsimd.dma_start(out=itT, in_=indT_d[:, ch * P:(ch + 1) * P])
        indT_t.append(itT)
    eps8 = const.tile([G, 1], F32, tag="eps8")
    nc.vector.memset(eps8, EPS)
    sqrt_warm = const.tile([G, 1], F32, tag="sqrt_warm")
    nc.scalar.activation(out=sqrt_warm, in_=eps8, func=AF.Sqrt, bias=eps8, scale=1.0)

    # HAM warm-up: ~24 back-to-back matmuls on a memset tile keep the PE
    # busy during the DMA/GroupNorm ramp so the activity monitor unthrottles
    # the clock (1.2 -> 2.4 GHz) before real matmuls arrive
    warm_in = const.tile([P, IH], VAL_DT, tag="warm_in")
    nc.vector.memset(warm_in, 1.0)
    wpsum = pp.tile([P, IH], F32, tag="ps")
    for _ in range(24):
        nc.tensor.matmul(wpsum, warm_in[:, 0:P], warm_in, start=True, stop=True)
    warm_sink = const.tile([P, 1], F32, tag="warm_sink")
    nc.vector.tensor_copy(out=warm_sink, in_=wpsum[:, 0:1])

    # ---- per-batch pipeline, software-pipelined across batches ----

    def emit_head(b, h_on_act=False):
        xt = st[b]["xt"]

        # -- GroupNorm statistics --
        # per-channel mean / E[x^2] over the 1024 free elements
        first_inst = [None]
        pcs = []
        for ch in range(NCH):
            stats = sm.tile([P, 2, 6], F32, tag="bnstats")
            for sg in range(2):
                i_ = nc.vector.bn_stats(out=stats[:, sg, :], in_=xt[ch][:, sg * 512:(sg + 1) * 512])
                if first_inst[0] is None:
                    first_inst[0] = i_
            mv = sm.tile([P, 2], F32, tag="mv")
            nc.vector.bn_aggr(out=mv, in_=stats)
            pc = sm.tile([P, 2], F32, tag=f"pc{ch}")
            nc.vector.tensor_copy(out=pc[:, 0:1], in_=mv[:, 0:1])
            nc.vector.scalar_tensor_tensor(out=pc[:, 1:2], in0=mv[:, 0:1],
                                           scalar=mv[:, 0:1], in1=mv[:, 1:2],
                                           op0=OP.mult, op1=OP.add)  # mean^2 + var
            pcs.append(pc)
        # group-reduce across the 32 channels of each group (partition axis)
        pg = pp.tile([G, 2], F32, tag="ps")
        for ch in range(NCH):
            nc.tensor.matmul(pg, ind_t[ch], pcs[ch], start=(ch == 0), stop=(ch == NCH - 1))
        br8 = sm.tile([G, 2], F32, tag="br8")   # [:,0]=mean_g  [:,1]=rstd_g
        nc.vector.tensor_scalar_mul(out=br8, in0=pg, scalar1=1.0 / 32.0)
        m2g = sm.tile([G, 1], F32, tag="m2g")
        nc.vector.tensor_mul(m2g, br8[:, 0:1], br8[:, 0:1])
        nc.vector.tensor_sub(br8[:, 1:2], br8[:, 1:2], m2g)    # var_g
        nc.scalar.activation(out=br8[:, 1:2], in_=br8[:, 1:2], func=AF.Sqrt, bias=eps8, scale=1.0)
        nc.vector.reciprocal(out=br8[:, 1:2], in_=br8[:, 1:2])

        # broadcast group stats back to channels, fold gn affine, normalize
        ht = []
        for ch in range(NCH):
            pbc = pp.tile([P, 2], F32, tag="ps")
            nc.tensor.matmul(pbc, indT_t[ch], br8)
            s_ = sm.tile([P, 1], F32, tag=f"s{ch}")
            t_ = sm.tile([P, 1], F32, tag=f"t{ch}")
            nc.vector.tensor_mul(s_, pbc[:, 1:2], gnw_t[ch])   # s = rstd * w
            nc.vector.scalar_tensor_tensor(out=t_, in0=pbc[:, 0:1], scalar=s_,
                                           in1=gnb_t[ch], op0=OP.mult,
                                           op1=OP.subtract)    # t = mean*s - b
            h_ = pools["hp"].tile([P, N], SIG_DT, name=f"ht{ch}", tag=f"ht{ch}")
            last_h = [None]
            if h_on_act:
                # h = Identity(x*s + (-t)): exact affine on the ScalarEngine
                nt = sm.tile([P, 1], F32, tag=f"nt{ch}")
                nc.vector.tensor_scalar_mul(out=nt, in0=t_, scalar1=-1.0)
                nc.scalar.activation(out=h_, in_=xt[ch], func=AF.Identity,
                                     bias=nt, scale=s_)
            else:
                last_h[0] = nc.vector.tensor_scalar(
                    out=h_, in0=xt[ch], scalar1=s_, scalar2=t_,
                    op0=OP.mult, op1=OP.subtract)  # x*s - t
            ht.append(h_)
        st[b]["ht"] = ht
        st[b]["head_first"] = first_inst[0]
        st[b]["head_last"] = last_h[0]

    def emit_qkv(b, q_on_act=False):
        ht = st[b]["ht"]
        # -- q, k projections: [C, N] = W^T.T @ h (+ bias during PSUM move) --
        # i-half-major so attention on i-half 0 starts after only 4 moves
        qt = [pools["qk"].tile([P, N], SIG_DT, name=f"qt{och}", tag=f"qt{och}")
              for och in range(NCH)]
        kt = [pools["qk"].tile([P, N], SIG_DT, name=f"kt{och}", tag=f"kt{och}")
              for och in range(NCH)]
        for ih in range(NIH):
            for wname, dst in (("q", qt), ("k", kt)):
                for och in range(NCH):
                    pq = pp.tile([P, IH], F32, tag="ps")
                    for cch in range(NCH):
                        nc.tensor.matmul(
                            pq,
                            r(wt[wname][cch][:, och * P:(och + 1) * P]),
                            r(ht[cch][:, ih * IH:(ih + 1) * IH]),
                            start=(cch == 0), stop=(cch == NCH - 1))
                    if wname == "k" or q_on_act:
                        nc.scalar.add(out=dst[och][:, ih * IH:(ih + 1) * IH],
                                      in_=pq, add=bt[wname][och])
                    else:
                        nc.vector.tensor_scalar_add(
                            out=dst[och][:, ih * IH:(ih + 1) * IH], in0=pq,
                            scalar1=bt[wname][och])

        # -- v, produced transposed: vT[n, o] = h[:, n].T @ wvT + 1 (x) bv --
        vt = []
        for j in range(NJ):
            pv = pp.tile([P, C], F32, tag="ps")
            for cch in range(NCH):
                nc.tensor.matmul(pv, r(ht[cch][:, j * P:(j + 1) * P]), r(wt["v"][cch]),
                                 start=(cch == 0), stop=False)
            nc.tensor.matmul(pv, r(ones_row), r(bv_row), start=False, stop=True)
            v_ = pools["vtp"].tile([P, C], VAL_DT, name=f"vt{j}", tag=f"vt{j}")
            nc.scalar.copy(out=v_, in_=pv)
            vt.append(v_)
        st[b].update(qt=qt, kt=kt, vt=vt)

    def emit_attn_scores_wide(b):
        qt, kt, vt = (st[b][k] for k in ("qt", "kt", "vt"))
        st[b]["fin"] = [pools["outp"].tile([P, N], F32, name=f"fin{och}",
                                           tag=f"fin{och}") for och in range(NCH)]
        ets = [pools["etp"].tile([P, N], VAL_DT, name=f"et{j}", tag=f"et{j}")
               for j in range(NJ)]
        prs = pp.tile([P, N], F32, name="prs", tag="acc_rs", bufs=1)
        po = [pp.tile([P, N], F32, name=f"po{i}", tag=f"acc{i}", bufs=1)
              for i in range(NCH)]

        def s_pair(j):
            for ih in range(NIH):
                isl = slice(ih * IH, (ih + 1) * IH)
                ps = pp.tile([P, IH], F32, tag="ps")
                for cch in range(NCH):
                    nc.tensor.matmul(ps,
                                     r(kt[cch][:, j * P:(j + 1) * P]),
                                     r(qt[cch][:, isl]),
                                     start=(cch == 0), stop=(cch == NCH - 1))
                nc.scalar.activation(out=ets[j][:, isl], in_=ps, func=AF.Exp)

        def acc(j):
            nc.tensor.matmul(prs, r(ones128), r(ets[j]),
                             start=(j == 0), stop=(j == NJ - 1))
            for och in range(NCH):
                nc.tensor.matmul(po[och], r(vt[j][:, och * P:(och + 1) * P]),
                                 r(ets[j]), start=(j == 0), stop=(j == NJ - 1))

        s_pair(0)
        for j in range(1, NJ):
            s_pair(j)
            acc(j - 1)
        acc(NJ - 1)
        st[b]["acc"] = (prs, po)

    def emit_attn_norm_wide(b):
        prs, po = st[b]["acc"]
        rb = sm.tile([P, N], F32, tag="rb", bufs=2)
        rscratch = sm.tile([P, N], F32, tag="rscratch", bufs=2)
        nc.vector.reciprocal_approx_accurate(out=rb, in_=prs, scratch=rscratch)
        ou = []
        for cch in range(NCH):
            o_ = pools["outp"].tile([P, N], VAL_DT, name=f"ou{cch}", tag=f"ou{cch}")
            nc.vector.tensor_mul(o_, po[cch], rb)
            ou.append(o_)
        st[b]["ou"] = ou

    def emit_attn_out_wide(b):
        xt, fin = st[b]["xt"], st[b]["fin"]
        ou = st[b]["ou"]
        for och in range(NCH):
            pz = pp.tile([P, N], F32, name=f"pz{och}", tag=f"acc{och}", bufs=1)
            for cch in range(NCH):
                nc.tensor.matmul(pz,
                                 r(wt["p"][cch][:, och * P:(och + 1) * P]),
                                 r(ou[cch]),
                                 start=(cch == 0), stop=(cch == NCH - 1))
            nc.vector.scalar_tensor_tensor(
                out=fin[och], in0=pz, scalar=bt["p"][och],
                in1=xt[och], op0=OP.add, op1=OP.add)

    def emit_attn_scores(b, ih):
        qt, kt, vt = (st[b][k] for k in ("qt", "kt", "vt"))
        if ih == 0:
            st[b]["fin"] = [pools["outp"].tile([P, N], F32, name=f"fin{och}",
                                               tag=f"fin{och}") for och in range(NCH)]
        isl = slice(ih * IH, (ih + 1) * IH)
        # rowsum replicated across all 128 partitions (all-ones stationary) so
        # the reciprocal runs wide and needs no partition broadcast
        prs = pp.tile([P, IH], F32, name="prs", tag="ps")
        po = [pp.tile([P, IH], F32, name=f"po{_}", tag="ps") for _ in range(NCH)]
        ets = [None] * NJ

        def s_stage(j):
            ps = pp.tile([P, IH], F32, tag="ps")
            for cch in range(NCH):
                nc.tensor.matmul(ps,
                                 r(kt[cch][:, j * P:(j + 1) * P]),
                                 r(qt[cch][:, isl]),
                                 start=(cch == 0), stop=(cch == NCH - 1))
            et = pools["etp"].tile([P, IH], VAL_DT, name=f"et{j}", tag=f"et{j}")
            nc.scalar.activation(out=et, in_=ps, func=AF.Exp)
            ets[j] = et

        def acc_stage(j):
            et = ets[j]
            nc.tensor.matmul(prs, r(ones128), r(et), start=(j == 0), stop=(j == NJ - 1))
            for och in range(NCH):
                nc.tensor.matmul(po[och], r(vt[j][:, och * P:(och + 1) * P]), r(et),
                                 start=(j == 0), stop=(j == NJ - 1))

        if PIPELINE_J:
            s_stage(0)
            for j in range(1, NJ):
                s_stage(j)
                acc_stage(j - 1)
            acc_stage(NJ - 1)
        else:
            for j in range(NJ):
                s_stage(j)
                acc_stage(j)
        st[b][f"acc{ih}"] = (prs, po)

    def emit_attn_norm(b, ih):
        prs, po = st[b][f"acc{ih}"]
        rb = sm.tile([P, IH], F32, tag="rb")
        rscratch = sm.tile([P, IH], F32, tag="rscratch")
        nc.vector.reciprocal_approx_accurate(out=rb, in_=prs, scratch=rscratch)
        ou = []
        for cch in range(NCH):
            o_ = pools["outp"].tile([P, IH], VAL_DT, name=f"ou{cch}", tag=f"ou{cch}")
            nc.vector.tensor_mul(o_, po[cch], rb)           # normalize
            ou.append(o_)
        st[b][f"ou{ih}"] = ou

    def emit_attn_out(b, ih):
        xt, fin = st[b]["xt"], st[b]["fin"]
        ou = st[b][f"ou{ih}"]
        isl = slice(ih * IH, (ih + 1) * IH)
        for och in range(NCH):
            pz = pp.tile([P, IH], F32, tag="ps")
            for cch in range(NCH):
                nc.tensor.matmul(pz,
                                 r(wt["p"][cch][:, och * P:(och + 1) * P]),
                                 r(ou[cch]),
                                 start=(cch == 0), stop=(cch == NCH - 1))
            # y = (wp@ou + bp) + x   in one fused DVE pass
            nc.vector.scalar_tensor_tensor(
                out=fin[och][:, isl], in0=pz, scalar=bt["p"][och],
                in1=xt[och][:, isl], op0=OP.add, op1=OP.add)
            nc.sync.dma_start(out=y_d[b, och * P:(och + 1) * P, isl],
                              in_=fin[och][:, isl])

    def emit_out(b):
        del st[b]

    # heads hoisted early (clusters ACT Sqrt table loads, frees DVE early);
    # qkv(0) right after head(0) so the PE has work during heads 1-3
    emit_head(0)
    emit_qkv(0)
    for b in range(1, BPC):
        emit_head(b)
        prev_last = st[b - 1]["head_last"]
        cur_first = st[b]["head_first"]
        if prev_last is not None and cur_first is not None:
            add_dep_helper(prev_last.ins if hasattr(prev_last, "ins") else prev_last,
                           cur_first.ins if hasattr(cur_first, "ins") else cur_first,
                           sync=False,
                           reason="head chain order: keep head(b-1) tail ahead of head(b) stats on DVE")
    for b in range(BPC):
        if WIDE_ACC:
            emit_attn_scores_wide(b)
            emit_attn_norm_wide(b)
            if b + 1 < BPC:
                emit_qkv(b + 1)
            emit_attn_out_wide(b)
            emit_out(b)
        else:
            emit_attn_scores(b, 0)
            emit_attn_norm(b, 0)
            emit_attn_scores(b, 1)
            emit_attn_out(b, 0)
            emit_attn_norm(b, 1)
            if b + 1 < BPC:
                emit_qkv(b + 1)
            emit_attn_out(b, 1)
            emit_out(b)

    for k in reversed(list(ctxpools)):
        ctxpools[k].__exit__(None, None, None)


def build_bass():
    nc = bacc.Bacc("TRN2", target_bir_lowering=False, debug=False)
    x_d = nc.dram_tensor("x", [BPC, C, N], F32, kind="ExternalInput")
    wd = {name: nc.dram_tensor(f"w{name}T", [C, C], VAL_DT if name == "p" else SIG_DT,
                               kind="ExternalInput")
          for name in ("q", "k", "v", "p")}
    bvr_d = nc.dram_tensor("bvr", [C], VAL_DT, kind="ExternalInput")
    spack_d = nc.dram_tensor("spack", [P, 26], F32, kind="ExternalInput")
    indT_d = nc.dram_tensor("indT", [G, C], F32, kind="ExternalInput")
    ones_d = nc.dram_tensor("ones", [P, P], VAL_DT, kind="ExternalInput")
    y_d = nc.dram_tensor("y", [BPC, C, N], F32, kind="ExternalOutput")

    with tile.TileContext(nc) as tc:
        build_kernel_body(nc, tc, x_d, y_d, wd, bvr_d, spack_d, indT_d, ones_d)
    nc.compile()
    return nc


def host_inputs(inputs):
    """Per-core replicated constants from the full input dict."""
    import ml_dtypes
    np_sig = np.float32 if SIG_DT != BF16 else ml_dtypes.bfloat16
    np_val = np.float32 if VAL_DT != BF16 else ml_dtypes.bfloat16
    f = lambda a: np.ascontiguousarray(np.asarray(a), dtype=np.float32)
    scale = np.float32(C ** -0.5)
    ind = np.zeros((C, G), dtype=np.float32)
    for c in range(C):
        ind[c, c // GS] = 1.0
    bq = f(inputs["bq"]) * scale
    bk = f(inputs["bk"])
    bp = f(inputs["bp"])
    gnw = f(inputs["gn_w"])
    gnb = f(inputs["gn_b"])
    spack = np.zeros((P, 26), dtype=np.float32)
    for ch in range(NCH):
        sl = slice(ch * P, (ch + 1) * P)
        spack[:, 0 + ch] = bq[sl]
        spack[:, 2 + ch] = bk[sl]
        spack[:, 4 + ch] = bp[sl]
        spack[:, 6 + ch] = gnw[sl]
        spack[:, 8 + ch] = gnb[sl]
        spack[:, 10 + 8 * ch:18 + 8 * ch] = ind[sl, :]
    consts = {
        "wqT": f(np.asarray(inputs["wq"], dtype=np.float32).T * scale).astype(np_sig),
        "wkT": f(np.asarray(inputs["wk"], dtype=np.float32).T).astype(np_sig),
        "wvT": f(np.asarray(inputs["wv"], dtype=np.float32).T).astype(np_sig),
        "wpT": f(np.asarray(inputs["wp"], dtype=np.float32).T).astype(np_val),
        "bvr": f(inputs["bv"]).astype(np_val),
        "spack": spack,
        "indT": np.ascontiguousarray(ind.T),
        "ones": np.ones((P, P), dtype=np_val),
    }
    return consts


_NC_CACHE = []


def _get_nc():
    if not _NC_CACHE:
        _NC_CACHE.append(build_bass())
    return _NC_CACHE[0]


def kernel(trace=False, trace_cores=None, **inputs):
    nc = _get_nc()
    consts = host_inputs(inputs)
    x = np.ascontiguousarray(np.asarray(inputs["x"], dtype=np.float32)).reshape(B, C, N)
    in_maps = []
    for core in range(NCORES):
        m = dict(consts)
        m["x"] = np.ascontiguousarray(x[core * BPC:(core + 1) * BPC])
        in_maps.append(m)
    res = run_bass_kernel_spmd(nc, in_maps, core_ids=list(range(NCORES)),
                               trace=trace, trace_cores=trace_cores)
    y = np.concatenate([r["y"] for r in res.results], axis=0)
    out = y.reshape(B, C, HH, WW).astype(np.float32)
    if trace:
        return out, res
    return out
